# revision 1
# baseline (speedup 1.0000x reference)
"""Bass/Trainium2 kernel for nn_FourierBlock (rfft -> per-mode complex einsum -> irfft).

Math (per head h, one head per NeuronCore, bf16 operands / f32 PSUM):
  X[m_ri, (b,i)]   = FB.T @ xT          forward DFT, 64 modes, chunk-outer
  Mst[i_ri, (b,m)] = per-b transposes   (concurrent tile-pair matmuls)
  O[o_ri, (m,b)]   = S_m.T @ Mst_m      per-mode stacked-complex einsum
  P_k[m_ri,(b2,o)] = per-b transposes   (concurrent tile-pair matmuls)
  Y[(b2,o), l]     = P_k.T @ G          inverse DFT per batch-pair k

Schedule notes (what measurably mattered on HW):
 - transposes are REGULAR matmuls (lhsT=data, rhs=identity) into f32 PSUM with
   explicit disjoint tile_position (0,0)/(64,64): the two quadrant matmuls run
   concurrently (~30ns/pair vs ~290ns serial transpose-mode pairs).
 - all critical inputs ride ONE DMA queue (sync) in consumption order; fb is
   packed in front of xq chunk 0 in a single transfer (one less issue at the
   queue head starts the whole drain train earlier). Compute chases chunks.
 - every DMA-gated stage starts ~1.5-2.5us after its data lands (HBM write-
   receipt latency before the semaphore fires); arrival-order consumption is
   provably optimal and the tax itself is irreducible.
 - outputs are written as four 512KB pairs; per-k f32->bf16 cast copies are
   split across vector+scalar (PSUM evacuation paces the tail; GPSIMD cannot
   read PSUM).
 - a dense 12-matmul warmup trips the PE clock gate (HAM) so the DMA-chased
   fwd DFT runs at 2.4GHz; mid-kernel re-warm bursts always lost (tested 4x),
   as did finer-grained copies, dual-queue issue, and fp8 operands (each fp8
   quantization alone costs 2-3% max-normalized error).
 - fewer, bigger ops win: per-op sequencer/semaphore overhead (~0.3-0.6us)
   dominates fine-grained variants.
"""

import numpy as np
import ml_dtypes

import concourse.bass as bass
import concourse.mybir as mybir
import concourse.tile as tile
from concourse import bacc
from concourse.bass_utils import run_bass_kernel_spmd
from concourse.masks import make_identity

B, L, H, E, M = 16, 1024, 8, 64, 64
BF = mybir.dt.bfloat16
F32 = mybir.dt.float32
NPBF = ml_dtypes.bfloat16

N_WARM = 12  # dummy 512-col matmuls to warm the PE clock before real work


def kernel_body(tc, outs, ins):
    nc = tc.nc
    ys = outs
    xq, s, g = ins

    with (
        tc.tile_pool(name="const", bufs=1) as const,
        tc.tile_pool(name="work", bufs=1) as work,
        tc.tile_pool(name="yout", bufs=3) as yout,
        tc.tile_pool(name="pf", bufs=2, space="PSUM") as pf,
        tc.tile_pool(name="pa", bufs=2, space="PSUM") as pa,
        tc.tile_pool(name="py", bufs=4, space="PSUM") as py,
    ):
        # ---- input DMAs: consumption order on sync; fb rides in front of
        # xq chunk 0 in one transfer (one less issue at the queue head) ----
        fxq_sb = const.tile([128, 9216], BF, tag="fxq")
        nc.sync.dma_start(fxq_sb[:, 0:2048], xq[:, 0:2048])
        for c in range(1, 8):
            nc.sync.dma_start(
                fxq_sb[:, 1024 + c * 1024 : 1024 + (c + 1) * 1024],
                xq[:, 1024 + c * 1024 : 1024 + (c + 1) * 1024],
            )
        s_sb = const.tile([128, 8192], BF, tag="s")
        for c in range(8):
            nc.sync.dma_start(s_sb[:, c * 1024 : (c + 1) * 1024], s[:, c * 1024 : (c + 1) * 1024])
        g_sb = const.tile([128, 1024], BF, tag="g")
        nc.scalar.dma_start(g_sb[:], g[:])

        scratch = const.tile([128, 512], BF, tag="scratch")
        nc.vector.memset(scratch[:], 0.0)
        ident = const.tile([128, 128], BF, tag="ident")
        make_identity(nc, ident[:])

        # ---- PE warmup: dummy matmuls on zeros ----
        wp = pf.tile([128, 512], F32, tag="fh")
        for i in range(N_WARM):
            nc.tensor.matmul(
                wp[:], scratch[:, 0:128], scratch[:], start=(i == 0), stop=(i == N_WARM - 1)
            )

        # ---- forward DFT: X[m_ri, (b,i)], chunk-outer to chase the xq DMA ----
        X_sb = work.tile([128, 1024], BF, tag="xsb")
        Xp0 = pf.tile([128, 512], F32, tag="fh")
        Xp1 = pf.tile([128, 512], F32, tag="fh")
        Xp = [Xp0, Xp1]
        for c in range(8):
            for half in (0, 1):
                nc.tensor.matmul(
                    Xp[half][:],
                    fxq_sb[:, c * 128 : (c + 1) * 128],
                    fxq_sb[:, 1024 + c * 1024 + half * 512 : 1024 + c * 1024 + (half + 1) * 512],
                    start=(c == 0),
                    stop=(c == 7),
                )
        nc.vector.tensor_copy(X_sb[:, 0:512], Xp[0][:])
        nc.scalar.copy(X_sb[:, 512:1024], Xp[1][:])

        # ---- stage A transposes: Mst[i_ri, (b,m)], with HAM-warming filler ----
        Mst = work.tile([128, 1024], BF, tag="mst")
        for half in (0, 1):
            Mp = pa.tile([128, 512], F32, tag="ah")
            for b in range(half * 8, half * 8 + 8):
                cols = slice(b * 64 - half * 512, (b + 1) * 64 - half * 512)
                icols = slice(b * 64, (b + 1) * 64)
                nc.tensor.matmul(
                    Mp[0:64, cols], X_sb[0:64, icols], ident[0:64, 0:64],
                    start=True, stop=True, tile_position=(0, 0),
                )
                nc.tensor.matmul(
                    Mp[64:128, cols], X_sb[64:128, icols], ident[64:128, 64:128],
                    start=True, stop=True, tile_position=(64, 64),
                )
            if half == 0:
                nc.vector.tensor_copy(Mst[:, 0:512], Mp[:])
            else:
                nc.scalar.copy(Mst[:, 512:1024], Mp[:])

        # ---- einsum: O[o_ri, (m,b)] fp8 in, bf16 PSUM out ----
        O_sb = work.tile([128, 1024], BF, tag="osb")
        for half in (0, 1):
            Op = pf.tile([128, 512], F32, tag="fh")
            for m in range(half * 32, half * 32 + 32):
                nc.tensor.matmul(
                    Op[:, m * 16 - half * 512 : (m + 1) * 16 - half * 512],
                    s_sb[:, m * 128 : (m + 1) * 128],
                    Mst[:, m : 1024 : 64],
                    start=True,
                    stop=True,
                )
            dst = O_sb.rearrange("p (b mm) -> p b mm", b=16)[
                :, :, half * 32 : (half + 1) * 32
            ]
            if half == 0:
                nc.vector.tensor_copy(dst, Op.rearrange("p (mm b) -> p b mm", b=16))
            else:
                nc.scalar.copy(dst, Op.rearrange("p (mm b) -> p b mm", b=16))

        # ---- stage B transposes (batched): lhsT_all[m_ri, (k, j, o)] ----
        lhsT_all = work.tile([128, 1024], BF, tag="lhsT")
        for half in (0, 1):
            Pp = pa.tile([128, 512], F32, tag="ah")
            for kk in range(half * 4, half * 4 + 4):
                for j in (0, 1):
                    b = 2 * kk + j
                    cols = slice(
                        kk * 128 + j * 64 - half * 512,
                        kk * 128 + (j + 1) * 64 - half * 512,
                    )
                    bcols = slice(b * 64, (b + 1) * 64)
                    nc.tensor.matmul(
                        Pp[0:64, cols], O_sb[0:64, bcols], ident[0:64, 0:64],
                        start=True, stop=True, tile_position=(0, 0),
                    )
                    nc.tensor.matmul(
                        Pp[64:128, cols], O_sb[64:128, bcols], ident[64:128, 64:128],
                        start=True, stop=True, tile_position=(64, 64),
                    )
            if half == 0:
                nc.vector.tensor_copy(lhsT_all[:, 0:512], Pp[:])
            else:
                nc.scalar.copy(lhsT_all[:, 512:1024], Pp[:])

        # ---- per-k: iDFT -> split cast copy -> grouped output DMA on sync.
        # Small first group (drain starts earliest) and small last group
        # (short final drain): [0], [1,2], [3,4], [5,6], [7].
        groups = [[0], [1, 2], [3, 4], [5, 6], [7]]
        gidx = {k: (gi, g.index(k)) for gi, g in enumerate(groups) for k in g}
        y_sb = None
        for k in range(8):
            gi, pos = gidx[k]
            if pos == 0:
                y_grp_sb = yout.tile([128, 2048], BF, tag="ysb")
                y_sb = y_grp_sb
            lk = lhsT_all[:, k * 128 : (k + 1) * 128]
            Yp0 = py.tile([128, 512], F32, tag="yp")
            Yp1 = py.tile([128, 512], F32, tag="yp")
            nc.tensor.matmul(Yp0[:], lk, g_sb[:, 0:512], start=True, stop=True)
            nc.tensor.matmul(Yp1[:], lk, g_sb[:, 512:1024], start=True, stop=True)
            off = pos * 1024
            nc.vector.tensor_copy(y_sb[:, off : off + 512], Yp0[:])
            nc.scalar.copy(y_sb[:, off + 512 : off + 1024], Yp1[:])
            if pos == len(groups[gi]) - 1:
                n = len(groups[gi]) * 1024
                nc.sync.dma_start(ys[gi][:], y_sb[:, 0:n])


def build_nc():
    nc = bacc.Bacc("TRN2", target_bir_lowering=False, debug=False, num_devices=8)
    xq = nc.dram_tensor("xq", [128, 9216], BF, kind="ExternalInput").ap()
    s = nc.dram_tensor("s", [128, 8192], BF, kind="ExternalInput").ap()
    g = nc.dram_tensor("g", [128, 1024], BF, kind="ExternalInput").ap()
    ysizes = [1024, 2048, 2048, 2048, 1024]
    ys = [
        nc.dram_tensor(f"y{k}", [128, ysizes[k]], BF, kind="ExternalOutput").ap()
        for k in range(5)
    ]
    with tile.TileContext(nc) as tc:
        kernel_body(tc, ys, [xq, s, g])
    nc.compile()
    return nc


def host_basis():
    l = np.arange(L, dtype=np.float64)[:, None]
    m = np.arange(M, dtype=np.float64)[None, :]
    ang = 2 * np.pi * l * m / L
    FB = np.concatenate([np.cos(ang), -np.sin(ang)], axis=1)  # [L, 128]
    c = np.full(M, 2.0)
    c[0] = 1.0
    GC = c[:, None] * np.cos(ang.T) / L
    GS = -c[:, None] * np.sin(ang.T) / L
    G = np.concatenate([GC, GS], axis=0)  # [128, L]
    # chunk-major layout for direct [128, ...] DMA
    fb_host = np.ascontiguousarray(
        FB.reshape(8, 128, 128).transpose(1, 0, 2).reshape(128, 1024)
    ).astype(NPBF)
    g_host = np.ascontiguousarray(G).astype(NPBF)
    return fb_host, g_host


def host_inputs(q, w_real, w_imag):
    fb_host, g_host = host_basis()
    in_maps = []
    for h in range(H):
        x = q[:, :, h, :]  # [B, L, E]
        xT = np.transpose(x, (1, 0, 2)).reshape(L, B * E)  # [L, (b,i)] b-major
        xq_host = np.concatenate(
            [
                fb_host,
                np.ascontiguousarray(
                    xT.reshape(8, 128, B * E).transpose(1, 0, 2).reshape(128, 8 * 1024)
                ).astype(NPBF),
            ],
            axis=1,
        )
        # einsum stationaries: S_m = [[Wr, Wi], [-Wi, Wr]]  (rows i_ri, cols o_ri)
        Wr = w_real[h].astype(np.float32)  # [i, o, m]
        Wi = w_imag[h].astype(np.float32)
        Sm = np.empty((M, 128, 128), dtype=np.float32)
        Sm[:, 0:64, 0:64] = Wr.transpose(2, 0, 1)
        Sm[:, 0:64, 64:128] = Wi.transpose(2, 0, 1)
        Sm[:, 64:128, 0:64] = -Wi.transpose(2, 0, 1)
        Sm[:, 64:128, 64:128] = Wr.transpose(2, 0, 1)
        s_host = np.ascontiguousarray(Sm.transpose(1, 0, 2).reshape(128, 8192)).astype(
            NPBF
        )
        in_maps.append({"xq": xq_host, "s": s_host, "g": g_host})
    return in_maps


def assemble(results):
    out = np.empty((B, H, E, L), dtype=np.float32)
    for h in range(H):
        parts = []
        for k in range(5):
            arr = results[h][f"y{k}"].astype(np.float32)
            n = arr.shape[1] // 1024
            parts += [arr.reshape(128, n, 1024)[:, j, :] for j in range(n)]
        yh = np.stack(parts)  # [k, 128, L]
        out[:, h, :, :] = yh.reshape(B, E, L)  # [(k,j)=b, o, l]
    return out


_NC_CACHE = {}


def run(q, w_real, w_imag, **kwargs):
    if "nc" not in _NC_CACHE:
        _NC_CACHE["nc"] = build_nc()
    nc = _NC_CACHE["nc"]
    in_maps = host_inputs(
        np.asarray(q, dtype=np.float32),
        np.asarray(w_real, dtype=np.float32),
        np.asarray(w_imag, dtype=np.float32),
    )
    res = run_bass_kernel_spmd(nc, in_maps, core_ids=list(range(H)), **kwargs)
    return assemble(res.results), res


def kernel(q, w_real, w_imag):
    return run(q, w_real, w_imag)[0]



# revision 7
# speedup vs baseline: 1.0034x; 1.0034x over previous
"""Bass/Trainium2 kernel for nn_FourierBlock (rfft -> per-mode complex einsum -> irfft).

Math (per head h, one head per NeuronCore, bf16 operands / f32 PSUM):
  X[m_ri, (b,i)]   = FB.T @ xT          forward DFT, 64 modes, chunk-outer
  Mst[i_ri, (b,m)] = per-b transposes   (concurrent tile-pair matmuls)
  O[o_ri, (m,b)]   = S_m.T @ Mst_m      per-mode stacked-complex einsum
  P_k[m_ri,(b2,o)] = per-b transposes   (concurrent tile-pair matmuls)
  Y[(b2,o), l]     = P_k.T @ G          inverse DFT per batch-pair k

Schedule notes (what measurably mattered on HW):
 - transposes are REGULAR matmuls (lhsT=data, rhs=identity) into f32 PSUM with
   explicit disjoint tile_position (0,0)/(64,64): the two quadrant matmuls run
   concurrently (~30ns/pair vs ~290ns serial transpose-mode pairs).
 - all critical inputs ride ONE DMA queue (sync) in consumption order; fb is
   packed in front of xq chunk 0 in a single transfer (one less issue at the
   queue head starts the whole drain train earlier). Compute chases chunks.
 - every DMA-gated stage starts ~1.5-2.5us after its data lands (HBM write-
   receipt latency before the semaphore fires); arrival-order consumption is
   provably optimal and the tax itself is irreducible.
 - outputs are written as four 512KB pairs; per-k f32->bf16 cast copies are
   split across vector+scalar (PSUM evacuation paces the tail; GPSIMD cannot
   read PSUM).
 - a dense 12-matmul warmup trips the PE clock gate (HAM) so the DMA-chased
   fwd DFT runs at 2.4GHz; mid-kernel re-warm bursts always lost (tested 4x),
   as did finer-grained copies, dual-queue issue, and fp8 operands (each fp8
   quantization alone costs 2-3% max-normalized error).
 - fewer, bigger ops win: per-op sequencer/semaphore overhead (~0.3-0.6us)
   dominates fine-grained variants.
"""

import numpy as np
import ml_dtypes

import concourse.bass as bass
import concourse.mybir as mybir
import concourse.tile as tile
from concourse import bacc
from concourse.bass_utils import run_bass_kernel_spmd
from concourse.masks import make_identity

B, L, H, E, M = 16, 1024, 8, 64, 64
BF = mybir.dt.bfloat16
F32 = mybir.dt.float32
NPBF = ml_dtypes.bfloat16

N_WARM = 12  # dummy 512-col matmuls to warm the PE clock before real work


def kernel_body(tc, outs, ins):
    nc = tc.nc
    ys = outs
    xq, s, g = ins

    with (
        tc.tile_pool(name="const", bufs=1) as const,
        tc.tile_pool(name="work", bufs=1) as work,
        tc.tile_pool(name="yout", bufs=8) as yout,
        tc.tile_pool(name="pf", bufs=2, space="PSUM") as pf,
        tc.tile_pool(name="pa", bufs=2, space="PSUM") as pa,
        tc.tile_pool(name="py", bufs=4, space="PSUM") as py,
    ):
        # ---- input DMAs: consumption order; the HEAD transfer (fb + xq chunk
        # 0) is issued from the SCALAR HWDGE ring because ACT's framework
        # prologue retires ~1us before SP's -- its packets hit HBM earlier and
        # pull the whole arrival train forward. Everything else rides sync in
        # consumption order (xq chunks, then s, then g). ----
        fxq_sb = const.tile([128, 9216], BF, tag="fxq")
        nc.scalar.dma_start(fxq_sb[:, 0:2048], xq[:, 0:2048])
        for c in range(1, 8):
            nc.sync.dma_start(
                fxq_sb[:, 1024 + c * 1024 : 1024 + (c + 1) * 1024],
                xq[:, 1024 + c * 1024 : 1024 + (c + 1) * 1024],
            )
        s_sb = const.tile([128, 8192], BF, tag="s")
        for c in range(4):
            nc.sync.dma_start(s_sb[:, c * 2048 : (c + 1) * 2048], s[:, c * 2048 : (c + 1) * 2048])
        g_sb = const.tile([128, 1024], BF, tag="g")
        nc.sync.dma_start(g_sb[:], g[:])

        scratch = const.tile([128, 512], BF, tag="scratch")
        nc.vector.memset(scratch[:], 0.0)
        ident = const.tile([128, 128], BF, tag="ident")
        make_identity(nc, ident[:])

        # ---- PE warmup: dummy matmuls on zeros ----
        wp = pf.tile([128, 512], F32, tag="fh")
        for i in range(N_WARM):
            nc.tensor.matmul(
                wp[:], scratch[:, 0:128], scratch[:], start=(i == 0), stop=(i == N_WARM - 1)
            )

        # ---- forward DFT: X[m_ri, (b,i)], chunk-outer to chase the xq DMA ----
        X_sb = work.tile([128, 1024], BF, tag="xsb")
        Xp0 = pf.tile([128, 512], F32, tag="fh")
        Xp1 = pf.tile([128, 512], F32, tag="fh")
        Xp = [Xp0, Xp1]
        for c in range(8):
            for half in (0, 1):
                nc.tensor.matmul(
                    Xp[half][:],
                    fxq_sb[:, c * 128 : (c + 1) * 128],
                    fxq_sb[:, 1024 + c * 1024 + half * 512 : 1024 + c * 1024 + (half + 1) * 512],
                    start=(c == 0),
                    stop=(c == 7),
                )
        nc.vector.tensor_copy(X_sb[:, 0:512], Xp[0][:])
        nc.scalar.copy(X_sb[:, 512:1024], Xp[1][:])

        # ---- stage A transposes: Mst[i_ri, (b,m)], with HAM-warming filler ----
        Mst = work.tile([128, 1024], BF, tag="mst")
        for half in (0, 1):
            Mp = pa.tile([128, 512], F32, tag="ah")
            for b in range(half * 8, half * 8 + 8):
                cols = slice(b * 64 - half * 512, (b + 1) * 64 - half * 512)
                icols = slice(b * 64, (b + 1) * 64)
                nc.tensor.matmul(
                    Mp[0:64, cols], X_sb[0:64, icols], ident[0:64, 0:64],
                    start=True, stop=True, tile_position=(0, 0),
                )
                nc.tensor.matmul(
                    Mp[64:128, cols], X_sb[64:128, icols], ident[64:128, 64:128],
                    start=True, stop=True, tile_position=(64, 64),
                )
            if half == 0:
                nc.vector.tensor_copy(Mst[:, 0:512], Mp[:])
            else:
                nc.scalar.copy(Mst[:, 512:1024], Mp[:])

        # ---- einsum: O[o_ri, (m,b)] ----
        # The einsum is s-ARRIVAL-paced (PE issue outruns the DMA 2.9x), so the
        # PE idles in the stall slots and HAM re-throttles the clock to 1.2GHz
        # right before the iDFT train. Dummy 512-col matmuls (into the py pool,
        # never read) fill those slots to hold HAM at 8/8 -- they cost nothing
        # on the critical path because they run while the PE waits for s bytes.
        def ham_filler():
            dm = py.tile([128, 512], F32, tag="yp")
            nc.tensor.matmul(dm[:], fxq_sb[:, 0:128], fxq_sb[:, 0:512], start=True, stop=True)

        O_sb = work.tile([128, 1024], BF, tag="osb")
        ham_filler()
        ham_filler()
        for half in (0, 1):
            Op = pf.tile([128, 512], F32, tag="fh")
            for m in range(half * 32, half * 32 + 32):
                nc.tensor.matmul(
                    Op[:, m * 16 - half * 512 : (m + 1) * 16 - half * 512],
                    s_sb[:, m * 128 : (m + 1) * 128],
                    Mst[:, m : 1024 : 64],
                    start=True,
                    stop=True,
                )
                if m % 8 == 7 and m < 60:
                    ham_filler()
            dst = O_sb.rearrange("p (b mm) -> p b mm", b=16)[
                :, :, half * 32 : (half + 1) * 32
            ]
            if half == 0:
                nc.vector.tensor_copy(dst, Op.rearrange("p (mm b) -> p b mm", b=16))
            else:
                nc.scalar.copy(dst, Op.rearrange("p (mm b) -> p b mm", b=16))

        # ---- stage B transposes (batched): lhsT_all[m_ri, (k, j, o)] ----
        lhsT_all = work.tile([128, 1024], BF, tag="lhsT")
        for half in (0, 1):
            Pp = pa.tile([128, 512], F32, tag="ah")
            for kk in range(half * 4, half * 4 + 4):
                for j in (0, 1):
                    b = 2 * kk + j
                    cols = slice(
                        kk * 128 + j * 64 - half * 512,
                        kk * 128 + (j + 1) * 64 - half * 512,
                    )
                    bcols = slice(b * 64, (b + 1) * 64)
                    nc.tensor.matmul(
                        Pp[0:64, cols], O_sb[0:64, bcols], ident[0:64, 0:64],
                        start=True, stop=True, tile_position=(0, 0),
                    )
                    nc.tensor.matmul(
                        Pp[64:128, cols], O_sb[64:128, bcols], ident[64:128, 64:128],
                        start=True, stop=True, tile_position=(64, 64),
                    )
            if half == 0:
                nc.vector.tensor_copy(lhsT_all[:, 0:512], Pp[:])
            else:
                nc.scalar.copy(lhsT_all[:, 512:1024], Pp[:])

        # ---- per-k: iDFT -> split cast copy -> per-k output DMA on sync.
        # One SBUF buffer per k (bufs=8): no buffer reuse, so no stall waiting
        # on an earlier group's HBM write-receipt before the copy can land.
        for k in range(8):
            y_sb = yout.tile([128, 1024], BF, tag="ysb")
            lk = lhsT_all[:, k * 128 : (k + 1) * 128]
            Yp0 = py.tile([128, 512], F32, tag="yp")
            Yp1 = py.tile([128, 512], F32, tag="yp")
            nc.tensor.matmul(Yp0[:], lk, g_sb[:, 0:512], start=True, stop=True)
            nc.tensor.matmul(Yp1[:], lk, g_sb[:, 512:1024], start=True, stop=True)
            nc.vector.tensor_copy(y_sb[:, 0:512], Yp0[:])
            nc.scalar.copy(y_sb[:, 512:1024], Yp1[:])
            nc.sync.dma_start(ys[k][:], y_sb[:])


def build_nc():
    nc = bacc.Bacc("TRN2", target_bir_lowering=False, debug=False, num_devices=8)
    xq = nc.dram_tensor("xq", [128, 9216], BF, kind="ExternalInput").ap()
    s = nc.dram_tensor("s", [128, 8192], BF, kind="ExternalInput").ap()
    g = nc.dram_tensor("g", [128, 1024], BF, kind="ExternalInput").ap()
    ys = [
        nc.dram_tensor(f"y{k}", [128, 1024], BF, kind="ExternalOutput").ap()
        for k in range(8)
    ]
    with tile.TileContext(nc) as tc:
        kernel_body(tc, ys, [xq, s, g])
    nc.compile()
    return nc


def host_basis():
    l = np.arange(L, dtype=np.float64)[:, None]
    m = np.arange(M, dtype=np.float64)[None, :]
    ang = 2 * np.pi * l * m / L
    FB = np.concatenate([np.cos(ang), -np.sin(ang)], axis=1)  # [L, 128]
    c = np.full(M, 2.0)
    c[0] = 1.0
    GC = c[:, None] * np.cos(ang.T) / L
    GS = -c[:, None] * np.sin(ang.T) / L
    G = np.concatenate([GC, GS], axis=0)  # [128, L]
    # chunk-major layout for direct [128, ...] DMA
    fb_host = np.ascontiguousarray(
        FB.reshape(8, 128, 128).transpose(1, 0, 2).reshape(128, 1024)
    ).astype(NPBF)
    g_host = np.ascontiguousarray(G).astype(NPBF)
    return fb_host, g_host


def host_inputs(q, w_real, w_imag):
    fb_host, g_host = host_basis()
    in_maps = []
    for h in range(H):
        x = q[:, :, h, :]  # [B, L, E]
        xT = np.transpose(x, (1, 0, 2)).reshape(L, B * E)  # [L, (b,i)] b-major
        xq_host = np.concatenate(
            [
                fb_host,
                np.ascontiguousarray(
                    xT.reshape(8, 128, B * E).transpose(1, 0, 2).reshape(128, 8 * 1024)
                ).astype(NPBF),
            ],
            axis=1,
        )
        # einsum stationaries: S_m = [[Wr, Wi], [-Wi, Wr]]  (rows i_ri, cols o_ri)
        Wr = w_real[h].astype(np.float32)  # [i, o, m]
        Wi = w_imag[h].astype(np.float32)
        Sm = np.empty((M, 128, 128), dtype=np.float32)
        Sm[:, 0:64, 0:64] = Wr.transpose(2, 0, 1)
        Sm[:, 0:64, 64:128] = Wi.transpose(2, 0, 1)
        Sm[:, 64:128, 0:64] = -Wi.transpose(2, 0, 1)
        Sm[:, 64:128, 64:128] = Wr.transpose(2, 0, 1)
        s_host = np.ascontiguousarray(Sm.transpose(1, 0, 2).reshape(128, 8192)).astype(
            NPBF
        )
        in_maps.append({"xq": xq_host, "s": s_host, "g": g_host})
    return in_maps


def assemble(results):
    out = np.empty((B, H, E, L), dtype=np.float32)
    for h in range(H):
        yh = np.stack([results[h][f"y{k}"].astype(np.float32) for k in range(8)])
        out[:, h, :, :] = yh.reshape(B, E, L)  # [(k,j)=b, o, l]
    return out


_NC_CACHE = {}


def run(q, w_real, w_imag, **kwargs):
    if "nc" not in _NC_CACHE:
        _NC_CACHE["nc"] = build_nc()
    nc = _NC_CACHE["nc"]
    in_maps = host_inputs(
        np.asarray(q, dtype=np.float32),
        np.asarray(w_real, dtype=np.float32),
        np.asarray(w_imag, dtype=np.float32),
    )
    res = run_bass_kernel_spmd(nc, in_maps, core_ids=list(range(H)), **kwargs)
    return assemble(res.results), res


def kernel(q, w_real, w_imag):
    return run(q, w_real, w_imag)[0]



# revision 12
# speedup vs baseline: 1.0162x; 1.0127x over previous
"""Bass/Trainium2 kernel for nn_FourierBlock (rfft -> per-mode complex einsum -> irfft).

Math (per head h, one head per NeuronCore, bf16 operands / f32 PSUM):
  X[m_ri, (b,i)]   = FB.T @ xT          forward DFT, 64 modes, chunk-outer
  Mst[i_ri, (b,m)] = per-b transposes   (concurrent tile-pair matmuls)
  O[o_ri, (m,b)]   = S_m.T @ Mst_m      per-mode stacked-complex einsum
  P_k[m_ri,(b2,o)] = per-b transposes   (concurrent tile-pair matmuls)
  Y[(b2,o), l]     = P_k.T @ G          inverse DFT per batch-pair k

Schedule notes (what measurably mattered on HW):
 - transposes are REGULAR matmuls (lhsT=data, rhs=identity) into f32 PSUM with
   explicit disjoint tile_position (0,0)/(64,64): the two quadrant matmuls run
   concurrently (~30ns/pair vs ~290ns serial transpose-mode pairs).
 - all critical inputs ride ONE DMA queue (sync) in consumption order; fb is
   packed in front of xq chunk 0 in a single transfer (one less issue at the
   queue head starts the whole drain train earlier). Compute chases chunks.
 - every DMA-gated stage starts ~1.5-2.5us after its data lands (HBM write-
   receipt latency before the semaphore fires); arrival-order consumption is
   provably optimal and the tax itself is irreducible.
 - outputs are written as four 512KB pairs; per-k f32->bf16 cast copies are
   split across vector+scalar (PSUM evacuation paces the tail; GPSIMD cannot
   read PSUM).
 - a dense 12-matmul warmup trips the PE clock gate (HAM) so the DMA-chased
   fwd DFT runs at 2.4GHz; mid-kernel re-warm bursts always lost (tested 4x),
   as did finer-grained copies, dual-queue issue, and fp8 operands (each fp8
   quantization alone costs 2-3% max-normalized error).
 - fewer, bigger ops win: per-op sequencer/semaphore overhead (~0.3-0.6us)
   dominates fine-grained variants.
"""

import numpy as np
import ml_dtypes

import concourse.bass as bass
import concourse.mybir as mybir
import concourse.tile as tile
from concourse import bacc
from concourse.bass_utils import run_bass_kernel_spmd
from concourse.masks import make_identity

B, L, H, E, M = 16, 1024, 8, 64, 64
BF = mybir.dt.bfloat16
F32 = mybir.dt.float32
NPBF = ml_dtypes.bfloat16

N_WARM = 9  # dummy 512-col matmuls to warm the PE clock before real work


def kernel_body(tc, outs, ins):
    nc = tc.nc
    ys = outs
    xq, s, g = ins

    with (
        tc.tile_pool(name="const", bufs=1) as const,
        tc.tile_pool(name="work", bufs=1) as work,
        tc.tile_pool(name="yout", bufs=8) as yout,
        tc.tile_pool(name="pf", bufs=2, space="PSUM") as pf,
        tc.tile_pool(name="pa", bufs=2, space="PSUM") as pa,
        tc.tile_pool(name="py", bufs=4, space="PSUM") as py,
    ):
        # ---- input DMAs: consumption order on the sync ring. Transfer size
        # dominates effective HBM bandwidth (0.3MB chunks ~250GB/s vs ~1.2MB
        # ~340GB/s), so xq rides in TWO ~1.2MB transfers and s in TWO 1.05MB
        # transfers aligned with the einsum mode-halves. g (needed only by the
        # iDFT) goes last. ----
        fxq_sb = const.tile([128, 9216], BF, tag="fxq")
        nc.sync.dma_start(fxq_sb[:, 0:5120], xq[:, 0:5120])
        nc.sync.dma_start(fxq_sb[:, 5120:9216], xq[:, 5120:9216])
        s_sb = const.tile([128, 8192], BF, tag="s")
        nc.sync.dma_start(s_sb[:, 0:4096], s[:, 0:4096])
        nc.sync.dma_start(s_sb[:, 4096:8192], s[:, 4096:8192])
        g_sb = const.tile([128, 1024], BF, tag="g")
        nc.sync.dma_start(g_sb[:], g[:])

        scratch = const.tile([128, 512], BF, tag="scratch")
        nc.vector.memset(scratch[:], 0.0)
        ident = const.tile([128, 128], BF, tag="ident")
        make_identity(nc, ident[:])

        # ---- PE warmup: dummy matmuls on zeros ----
        wp = pf.tile([128, 512], F32, tag="fh")
        for i in range(N_WARM):
            nc.tensor.matmul(
                wp[:], scratch[:, 0:128], scratch[:], start=(i == 0), stop=(i == N_WARM - 1)
            )

        # ---- forward DFT: X[m_ri, (b,i)], chunk-outer to chase the xq DMA ----
        X_sb = work.tile([128, 1024], BF, tag="xsb")
        Xp0 = pf.tile([128, 512], F32, tag="fh")
        Xp1 = pf.tile([128, 512], F32, tag="fh")
        Xp = [Xp0, Xp1]
        for c in range(8):
            for half in (0, 1):
                nc.tensor.matmul(
                    Xp[half][:],
                    fxq_sb[:, c * 128 : (c + 1) * 128],
                    fxq_sb[:, 1024 + c * 1024 + half * 512 : 1024 + c * 1024 + (half + 1) * 512],
                    start=(c == 0),
                    stop=(c == 7),
                )
        nc.vector.tensor_copy(X_sb[:, 0:512], Xp[0][:])
        nc.scalar.copy(X_sb[:, 512:1024], Xp[1][:])

        # ---- stage A transposes: Mst[i_ri, (b,m)], with HAM-warming filler ----
        Mst = work.tile([128, 1024], BF, tag="mst")
        for half in (0, 1):
            Mp = pa.tile([128, 512], F32, tag="ah")
            for b in range(half * 8, half * 8 + 8):
                cols = slice(b * 64 - half * 512, (b + 1) * 64 - half * 512)
                icols = slice(b * 64, (b + 1) * 64)
                nc.tensor.matmul(
                    Mp[0:64, cols], X_sb[0:64, icols], ident[0:64, 0:64],
                    start=True, stop=True, tile_position=(0, 0),
                )
                nc.tensor.matmul(
                    Mp[64:128, cols], X_sb[64:128, icols], ident[64:128, 64:128],
                    start=True, stop=True, tile_position=(64, 64),
                )
            if half == 0:
                nc.vector.tensor_copy(Mst[:, 0:512], Mp[:])
            else:
                nc.scalar.copy(Mst[:, 512:1024], Mp[:])

        # ---- einsum: O[o_ri, (m,b)] ----
        # The einsum is s-ARRIVAL-paced (PE issue outruns the DMA 2.9x), so the
        # PE idles in the stall slots and HAM re-throttles the clock to 1.2GHz
        # right before the iDFT train. Dummy 512-col matmuls (into the py pool,
        # never read) fill those slots to hold HAM at 8/8 -- they cost nothing
        # on the critical path because they run while the PE waits for s bytes.
        def ham_filler():
            dm = py.tile([128, 512], F32, tag="yp")
            nc.tensor.matmul(dm[:], fxq_sb[:, 0:128], fxq_sb[:, 0:512], start=True, stop=True)

        O_sb = work.tile([128, 1024], BF, tag="osb")
        ham_filler()
        for half in (0, 1):
            Op = pf.tile([128, 512], F32, tag="fh")
            for m in range(half * 32, half * 32 + 32):
                nc.tensor.matmul(
                    Op[:, m * 16 - half * 512 : (m + 1) * 16 - half * 512],
                    s_sb[:, m * 128 : (m + 1) * 128],
                    Mst[:, m : 1024 : 64],
                    start=True,
                    stop=True,
                )
                if m % 16 == 15 and m < 60:
                    ham_filler()
            dst = O_sb.rearrange("p (b mm) -> p b mm", b=16)[
                :, :, half * 32 : (half + 1) * 32
            ]
            if half == 0:
                nc.vector.tensor_copy(dst, Op.rearrange("p (mm b) -> p b mm", b=16))
            else:
                nc.scalar.copy(dst, Op.rearrange("p (mm b) -> p b mm", b=16))

        # ---- stage B transposes (batched): lhsT_all[m_ri, (k, j, o)] ----
        lhsT_all = work.tile([128, 1024], BF, tag="lhsT")
        for half in (0, 1):
            Pp = pa.tile([128, 512], F32, tag="ah")
            for kk in range(half * 4, half * 4 + 4):
                for j in (0, 1):
                    b = 2 * kk + j
                    cols = slice(
                        kk * 128 + j * 64 - half * 512,
                        kk * 128 + (j + 1) * 64 - half * 512,
                    )
                    bcols = slice(b * 64, (b + 1) * 64)
                    nc.tensor.matmul(
                        Pp[0:64, cols], O_sb[0:64, bcols], ident[0:64, 0:64],
                        start=True, stop=True, tile_position=(0, 0),
                    )
                    nc.tensor.matmul(
                        Pp[64:128, cols], O_sb[64:128, bcols], ident[64:128, 64:128],
                        start=True, stop=True, tile_position=(64, 64),
                    )
            ham_filler()
            if half == 0:
                nc.vector.tensor_copy(lhsT_all[:, 0:512], Pp[:])
            else:
                nc.scalar.copy(lhsT_all[:, 512:1024], Pp[:])

        # ---- per-k: iDFT -> split cast copy -> per-k output DMA on sync.
        # One SBUF buffer per k (bufs=8): no buffer reuse, so no stall waiting
        # on an earlier group's HBM write-receipt before the copy can land.
        for k in range(8):
            y_sb = yout.tile([128, 1024], BF, tag="ysb")
            lk = lhsT_all[:, k * 128 : (k + 1) * 128]
            Yp0 = py.tile([128, 512], F32, tag="yp")
            Yp1 = py.tile([128, 512], F32, tag="yp")
            nc.tensor.matmul(Yp0[:], lk, g_sb[:, 0:512], start=True, stop=True)
            nc.tensor.matmul(Yp1[:], lk, g_sb[:, 512:1024], start=True, stop=True)
            nc.vector.tensor_copy(y_sb[:, 0:512], Yp0[:])
            nc.scalar.copy(y_sb[:, 512:1024], Yp1[:])
            nc.sync.dma_start(ys[k][:], y_sb[:])


def build_nc():
    nc = bacc.Bacc("TRN2", target_bir_lowering=False, debug=False, num_devices=8)
    xq = nc.dram_tensor("xq", [128, 9216], BF, kind="ExternalInput").ap()
    s = nc.dram_tensor("s", [128, 8192], BF, kind="ExternalInput").ap()
    g = nc.dram_tensor("g", [128, 1024], BF, kind="ExternalInput").ap()
    ys = [
        nc.dram_tensor(f"y{k}", [128, 1024], BF, kind="ExternalOutput").ap()
        for k in range(8)
    ]
    with tile.TileContext(nc) as tc:
        kernel_body(tc, ys, [xq, s, g])
    nc.compile()
    return nc


def host_basis():
    l = np.arange(L, dtype=np.float64)[:, None]
    m = np.arange(M, dtype=np.float64)[None, :]
    ang = 2 * np.pi * l * m / L
    FB = np.concatenate([np.cos(ang), -np.sin(ang)], axis=1)  # [L, 128]
    c = np.full(M, 2.0)
    c[0] = 1.0
    GC = c[:, None] * np.cos(ang.T) / L
    GS = -c[:, None] * np.sin(ang.T) / L
    G = np.concatenate([GC, GS], axis=0)  # [128, L]
    # chunk-major layout for direct [128, ...] DMA
    fb_host = np.ascontiguousarray(
        FB.reshape(8, 128, 128).transpose(1, 0, 2).reshape(128, 1024)
    ).astype(NPBF)
    g_host = np.ascontiguousarray(G).astype(NPBF)
    return fb_host, g_host


def host_inputs(q, w_real, w_imag):
    fb_host, g_host = host_basis()
    in_maps = []
    for h in range(H):
        x = q[:, :, h, :]  # [B, L, E]
        xT = np.transpose(x, (1, 0, 2)).reshape(L, B * E)  # [L, (b,i)] b-major
        xq_host = np.concatenate(
            [
                fb_host,
                np.ascontiguousarray(
                    xT.reshape(8, 128, B * E).transpose(1, 0, 2).reshape(128, 8 * 1024)
                ).astype(NPBF),
            ],
            axis=1,
        )
        # einsum stationaries: S_m = [[Wr, Wi], [-Wi, Wr]]  (rows i_ri, cols o_ri)
        Wr = w_real[h].astype(np.float32)  # [i, o, m]
        Wi = w_imag[h].astype(np.float32)
        Sm = np.empty((M, 128, 128), dtype=np.float32)
        Sm[:, 0:64, 0:64] = Wr.transpose(2, 0, 1)
        Sm[:, 0:64, 64:128] = Wi.transpose(2, 0, 1)
        Sm[:, 64:128, 0:64] = -Wi.transpose(2, 0, 1)
        Sm[:, 64:128, 64:128] = Wr.transpose(2, 0, 1)
        s_host = np.ascontiguousarray(Sm.transpose(1, 0, 2).reshape(128, 8192)).astype(
            NPBF
        )
        in_maps.append({"xq": xq_host, "s": s_host, "g": g_host})
    return in_maps


def assemble(results):
    out = np.empty((B, H, E, L), dtype=np.float32)
    for h in range(H):
        yh = np.stack([results[h][f"y{k}"].astype(np.float32) for k in range(8)])
        out[:, h, :, :] = yh.reshape(B, E, L)  # [(k,j)=b, o, l]
    return out


_NC_CACHE = {}


def run(q, w_real, w_imag, **kwargs):
    if "nc" not in _NC_CACHE:
        _NC_CACHE["nc"] = build_nc()
    nc = _NC_CACHE["nc"]
    in_maps = host_inputs(
        np.asarray(q, dtype=np.float32),
        np.asarray(w_real, dtype=np.float32),
        np.asarray(w_imag, dtype=np.float32),
    )
    res = run_bass_kernel_spmd(nc, in_maps, core_ids=list(range(H)), **kwargs)
    return assemble(res.results), res


def kernel(q, w_real, w_imag):
    return run(q, w_real, w_imag)[0]



# revision 15
# speedup vs baseline: 1.1360x; 1.1179x over previous
"""Bass/Trainium2 kernel for nn_FourierBlock (rfft -> per-mode complex einsum -> irfft).

Math (per head h, one head per NeuronCore, bf16 operands / f32 PSUM):
  X[m_ri, (b,i)]   = FB.T @ xT          forward DFT, 64 modes, chunk-outer
  Mst[i_ri, (b,m)] = per-b transposes   (concurrent tile-pair matmuls)
  O[o_ri, (m,b)]   = S_m.T @ Mst_m      per-mode stacked-complex einsum
  P_k[m_ri,(b2,o)] = per-b transposes   (concurrent tile-pair matmuls)
  Y[(b2,o), l]     = P_k.T @ G          inverse DFT per batch-pair k

Schedule notes (what measurably mattered on HW):
 - transposes are REGULAR matmuls (lhsT=data, rhs=identity) into f32 PSUM with
   explicit disjoint tile_position (0,0)/(64,64): the two quadrant matmuls run
   concurrently (~30ns/pair vs ~290ns serial transpose-mode pairs).
 - all critical inputs ride ONE DMA queue (sync) in consumption order; fb is
   packed in front of xq chunk 0 in a single transfer (one less issue at the
   queue head starts the whole drain train earlier). Compute chases chunks.
 - every DMA-gated stage starts ~1.5-2.5us after its data lands (HBM write-
   receipt latency before the semaphore fires); arrival-order consumption is
   provably optimal and the tax itself is irreducible.
 - outputs are written as four 512KB pairs; per-k f32->bf16 cast copies are
   split across vector+scalar (PSUM evacuation paces the tail; GPSIMD cannot
   read PSUM).
 - a dense 12-matmul warmup trips the PE clock gate (HAM) so the DMA-chased
   fwd DFT runs at 2.4GHz; mid-kernel re-warm bursts always lost (tested 4x),
   as did finer-grained copies, dual-queue issue, and fp8 operands (each fp8
   quantization alone costs 2-3% max-normalized error).
 - fewer, bigger ops win: per-op sequencer/semaphore overhead (~0.3-0.6us)
   dominates fine-grained variants.
"""

import numpy as np
import ml_dtypes

import concourse.bass as bass
import concourse.mybir as mybir
import concourse.tile as tile
from concourse import bacc
from concourse.bass_utils import run_bass_kernel_spmd
from concourse.masks import make_identity

B, L, H, E, M = 16, 1024, 8, 64, 64
BF = mybir.dt.bfloat16
F32 = mybir.dt.float32
NPBF = ml_dtypes.bfloat16

N_WARM = 9  # dummy 512-col matmuls to warm the PE clock before real work


def kernel_body(tc, outs, ins):
    nc = tc.nc
    ys = outs
    xq, s, g = ins

    with (
        tc.tile_pool(name="const", bufs=1) as const,
        tc.tile_pool(name="work", bufs=1) as work,
        tc.tile_pool(name="yout", bufs=8) as yout,
        tc.tile_pool(name="pf", bufs=2, space="PSUM") as pf,
        tc.tile_pool(name="pa", bufs=2, space="PSUM") as pa,
        tc.tile_pool(name="py", bufs=4, space="PSUM") as py,
    ):
        # ---- input DMAs: consumption order on the sync ring. Transfer size
        # dominates effective HBM bandwidth (0.3MB chunks ~250GB/s vs ~1.2MB
        # ~340GB/s), so xq rides in TWO ~1.2MB transfers and s in TWO 1.05MB
        # transfers aligned with the einsum mode-halves. g (needed only by the
        # iDFT) goes last. ----
        # A transfer's completion SEMAPHORE fires ~2.1us after its last byte
        # lands (write-receipt). The tail transfer of each tensor is kept
        # small so almost all DFT/einsum work is gated by sems that fired
        # while earlier bytes were still in flight.
        fxq_sb = const.tile([128, 9216], BF, tag="fxq")
        nc.sync.dma_start(fxq_sb[:, 0:5120], xq[:, 0:5120])
        nc.sync.dma_start(fxq_sb[:, 5120:8192], xq[:, 5120:8192])
        nc.sync.dma_start(fxq_sb[:, 8192:9216], xq[:, 8192:9216])
        s_sb = const.tile([128, 8192], BF, tag="s")
        nc.sync.dma_start(s_sb[:, 0:4096], s[:, 0:4096])
        nc.sync.dma_start(s_sb[:, 4096:6144], s[:, 4096:6144])
        nc.sync.dma_start(s_sb[:, 6144:8192], s[:, 6144:8192])
        g_sb = const.tile([128, 1024], BF, tag="g")
        nc.sync.dma_start(g_sb[:], g[:])

        scratch = const.tile([128, 512], BF, tag="scratch")
        nc.vector.memset(scratch[:], 0.0)
        ident = const.tile([128, 128], BF, tag="ident")
        make_identity(nc, ident[:])

        # ---- PE warmup: dummy matmuls on zeros ----
        wp = pf.tile([128, 512], F32, tag="fh")
        for i in range(N_WARM):
            nc.tensor.matmul(
                wp[:], scratch[:, 0:128], scratch[:], start=(i == 0), stop=(i == N_WARM - 1)
            )

        # ---- forward DFT: X[m_ri, (b,i)], chunk-outer to chase the xq DMA ----
        X_sb = work.tile([128, 1024], BF, tag="xsb")
        Xp0 = pf.tile([128, 512], F32, tag="fh")
        Xp1 = pf.tile([128, 512], F32, tag="fh")
        Xp = [Xp0, Xp1]
        for c in range(8):
            for half in (0, 1):
                nc.tensor.matmul(
                    Xp[half][:],
                    fxq_sb[:, c * 128 : (c + 1) * 128],
                    fxq_sb[:, 1024 + c * 1024 + half * 512 : 1024 + c * 1024 + (half + 1) * 512],
                    start=(c == 0),
                    stop=(c == 7),
                )
        nc.vector.tensor_copy(X_sb[:, 0:512], Xp[0][:])
        nc.scalar.copy(X_sb[:, 512:1024], Xp[1][:])

        # ---- stage A transposes: Mst[i_ri, (b,m)], with HAM-warming filler ----
        Mst = work.tile([128, 1024], BF, tag="mst")
        for half in (0, 1):
            Mp = pa.tile([128, 512], F32, tag="ah")
            for b in range(half * 8, half * 8 + 8):
                cols = slice(b * 64 - half * 512, (b + 1) * 64 - half * 512)
                icols = slice(b * 64, (b + 1) * 64)
                nc.tensor.matmul(
                    Mp[0:64, cols], X_sb[0:64, icols], ident[0:64, 0:64],
                    start=True, stop=True, tile_position=(0, 0),
                )
                nc.tensor.matmul(
                    Mp[64:128, cols], X_sb[64:128, icols], ident[64:128, 64:128],
                    start=True, stop=True, tile_position=(64, 64),
                )
            if half == 0:
                nc.vector.tensor_copy(Mst[:, 0:512], Mp[:])
            else:
                # h1 gates the einsum start: split across both engines
                nc.vector.tensor_copy(Mst[:, 512:768], Mp[:, 0:256])
                nc.scalar.copy(Mst[:, 768:1024], Mp[:, 256:512])

        # ---- einsum: O[o_ri, (m,b)] ----
        O_sb = work.tile([128, 1024], BF, tag="osb")
        for half in (0, 1):
            Op = pf.tile([128, 512], F32, tag="fh")
            for m in range(half * 32, half * 32 + 32):
                nc.tensor.matmul(
                    Op[:, m * 16 - half * 512 : (m + 1) * 16 - half * 512],
                    s_sb[:, m * 128 : (m + 1) * 128],
                    Mst[:, m : 1024 : 64],
                    start=True,
                    stop=True,
                )
            dst = O_sb.rearrange("p (b mm) -> p b mm", b=16)[
                :, :, half * 32 : (half + 1) * 32
            ]
            src = Op.rearrange("p (mm b) -> p b mm", b=16)
            if half == 0:
                nc.vector.tensor_copy(dst, src)
            else:
                # h1 gates trB: split across both engines (b 0-7 / b 8-15)
                nc.vector.tensor_copy(dst[:, 0:8], src[:, 0:8])
                nc.scalar.copy(dst[:, 8:16], src[:, 8:16])

        # ---- stage B transposes (batched): lhsT_all[m_ri, (k, j, o)] ----
        lhsT_all = work.tile([128, 1024], BF, tag="lhsT")
        for half in (0, 1):
            Pp = pa.tile([128, 512], F32, tag="ah")
            for kk in range(half * 4, half * 4 + 4):
                for j in (0, 1):
                    b = 2 * kk + j
                    cols = slice(
                        kk * 128 + j * 64 - half * 512,
                        kk * 128 + (j + 1) * 64 - half * 512,
                    )
                    bcols = slice(b * 64, (b + 1) * 64)
                    nc.tensor.matmul(
                        Pp[0:64, cols], O_sb[0:64, bcols], ident[0:64, 0:64],
                        start=True, stop=True, tile_position=(0, 0),
                    )
                    nc.tensor.matmul(
                        Pp[64:128, cols], O_sb[64:128, bcols], ident[64:128, 64:128],
                        start=True, stop=True, tile_position=(64, 64),
                    )
            if half == 0:
                # h0 gates iDFT k=0..3: split across both engines
                nc.vector.tensor_copy(lhsT_all[:, 0:256], Pp[:, 0:256])
                nc.scalar.copy(lhsT_all[:, 256:512], Pp[:, 256:512])
            else:
                nc.vector.tensor_copy(lhsT_all[:, 512:768], Pp[:, 0:256])
                nc.scalar.copy(lhsT_all[:, 768:1024], Pp[:, 256:512])

        # ---- per-k: iDFT -> split cast copy -> per-k output DMA on sync.
        # One SBUF buffer per k (bufs=8): no buffer reuse, so no stall waiting
        # on an earlier group's HBM write-receipt before the copy can land.
        for k in range(8):
            y_sb = yout.tile([128, 1024], BF, tag="ysb")
            lk = lhsT_all[:, k * 128 : (k + 1) * 128]
            Yp0 = py.tile([128, 512], F32, tag="yp")
            Yp1 = py.tile([128, 512], F32, tag="yp")
            nc.tensor.matmul(Yp0[:], lk, g_sb[:, 0:512], start=True, stop=True)
            nc.tensor.matmul(Yp1[:], lk, g_sb[:, 512:1024], start=True, stop=True)
            nc.vector.tensor_copy(y_sb[:, 0:512], Yp0[:])
            nc.scalar.copy(y_sb[:, 512:1024], Yp1[:])
            nc.sync.dma_start(ys[k][:], y_sb[:])


def build_nc():
    nc = bacc.Bacc("TRN2", target_bir_lowering=False, debug=False, num_devices=8)
    xq = nc.dram_tensor("xq", [128, 9216], BF, kind="ExternalInput").ap()
    s = nc.dram_tensor("s", [128, 8192], BF, kind="ExternalInput").ap()
    g = nc.dram_tensor("g", [128, 1024], BF, kind="ExternalInput").ap()
    ys = [
        nc.dram_tensor(f"y{k}", [128, 1024], BF, kind="ExternalOutput").ap()
        for k in range(8)
    ]
    with tile.TileContext(nc) as tc:
        kernel_body(tc, ys, [xq, s, g])
    nc.compile()
    return nc


def host_basis():
    l = np.arange(L, dtype=np.float64)[:, None]
    m = np.arange(M, dtype=np.float64)[None, :]
    ang = 2 * np.pi * l * m / L
    FB = np.concatenate([np.cos(ang), -np.sin(ang)], axis=1)  # [L, 128]
    c = np.full(M, 2.0)
    c[0] = 1.0
    GC = c[:, None] * np.cos(ang.T) / L
    GS = -c[:, None] * np.sin(ang.T) / L
    G = np.concatenate([GC, GS], axis=0)  # [128, L]
    # chunk-major layout for direct [128, ...] DMA
    fb_host = np.ascontiguousarray(
        FB.reshape(8, 128, 128).transpose(1, 0, 2).reshape(128, 1024)
    ).astype(NPBF)
    g_host = np.ascontiguousarray(G).astype(NPBF)
    return fb_host, g_host


def host_inputs(q, w_real, w_imag):
    fb_host, g_host = host_basis()
    in_maps = []
    for h in range(H):
        x = q[:, :, h, :]  # [B, L, E]
        xT = np.transpose(x, (1, 0, 2)).reshape(L, B * E)  # [L, (b,i)] b-major
        xq_host = np.concatenate(
            [
                fb_host,
                np.ascontiguousarray(
                    xT.reshape(8, 128, B * E).transpose(1, 0, 2).reshape(128, 8 * 1024)
                ).astype(NPBF),
            ],
            axis=1,
        )
        # einsum stationaries: S_m = [[Wr, Wi], [-Wi, Wr]]  (rows i_ri, cols o_ri)
        Wr = w_real[h].astype(np.float32)  # [i, o, m]
        Wi = w_imag[h].astype(np.float32)
        Sm = np.empty((M, 128, 128), dtype=np.float32)
        Sm[:, 0:64, 0:64] = Wr.transpose(2, 0, 1)
        Sm[:, 0:64, 64:128] = Wi.transpose(2, 0, 1)
        Sm[:, 64:128, 0:64] = -Wi.transpose(2, 0, 1)
        Sm[:, 64:128, 64:128] = Wr.transpose(2, 0, 1)
        s_host = np.ascontiguousarray(Sm.transpose(1, 0, 2).reshape(128, 8192)).astype(
            NPBF
        )
        in_maps.append({"xq": xq_host, "s": s_host, "g": g_host})
    return in_maps


def assemble(results):
    out = np.empty((B, H, E, L), dtype=np.float32)
    for h in range(H):
        yh = np.stack([results[h][f"y{k}"].astype(np.float32) for k in range(8)])
        out[:, h, :, :] = yh.reshape(B, E, L)  # [(k,j)=b, o, l]
    return out


_NC_CACHE = {}


def run(q, w_real, w_imag, **kwargs):
    if "nc" not in _NC_CACHE:
        _NC_CACHE["nc"] = build_nc()
    nc = _NC_CACHE["nc"]
    in_maps = host_inputs(
        np.asarray(q, dtype=np.float32),
        np.asarray(w_real, dtype=np.float32),
        np.asarray(w_imag, dtype=np.float32),
    )
    res = run_bass_kernel_spmd(nc, in_maps, core_ids=list(range(H)), **kwargs)
    return assemble(res.results), res


def kernel(q, w_real, w_imag):
    return run(q, w_real, w_imag)[0]

